# revision 1
# baseline (speedup 1.0000x reference)
"""Trainium2 Bass kernel: causal sliding-window attention + output projection.

Reference computation (B=1, H=16, T=2048, D=64, WINDOW=256, DIM=1024):
    att  = softmax(mask(q @ k^T / sqrt(D)))       per head, sliding causal window
    y    = att @ v                                 -> (B, H, T, D)
    out  = y.transpose -> (B, T, H*D) @ W_proj     -> (B, T, DIM)

Sharding over 8 NeuronCores: 2 head-groups (R) x 4 sequence-blocks (S).
Core c = (r, s): heads [8r, 8r+8), queries [512s, 512s+512), key window
[512s-256, 512s+512) (zero-padded below 0).  W_proj row-sharded per head
group; host sums the two partial projections per sequence block
(the "all-reduce after projection" done at gather time).

On-device layout (everything transposed so no on-chip transposes needed):
  scores^T[k, q] = (kT_ext)^T @ qT_ext      (65-row contraction: 64 dims +
                                             bias row giving -1e9 on padded keys)
  P^T = exp(scores * 1/8)                   one ACT op over [128, 1536]
  one full-width [128, 1536] tri-mask multiply zeroes out-of-window entries
  O = [ones | v]^T @ P^T                    -> [128, q]: rows 0:64 = softmax
                                               denom (replicated x64, so the
                                               custom reciprocal runs at base
                                               partition 0), rows 64:128 = y^T
  yT = O[64:128] * recip(O[0:64])           per head, into [128,512] chunks
  out[q, n] = sum_hp yT_hp^T @ W_hp         accumulated in PSUM, bf16 out
"""

import math
import os
import sys
from contextlib import ExitStack

import numpy as np

for _p in ("/opt/trn_rl_repo",):
    if _p not in sys.path and os.path.isdir(_p):
        sys.path.insert(0, _p)

import ml_dtypes  # noqa: E402

BF16NP = ml_dtypes.bfloat16

B, H, T, D = 1, 16, 2048, 64
DIM = H * D
WINDOW = 256
R, S = 2, 4                 # head groups x sequence blocks
HL = H // R                 # 8 heads per core
QL = T // S                 # 512 queries per core
KW = QL + WINDOW            # 768-key window per core
NKT = KW // 128             # 6 key tiles
NQB = QL // 128             # 4 query blocks
NEG = -1.0e9                # additive bias for padded (out-of-range) keys

# QK matmul pieces: (kt, score_col, q_col, width, start, stop).
# Scores live in one [128, 1536] fp32 PSUM tensor (3 banks of 512 f32 cols).
# Pieces are split so no matmul output crosses a bank boundary; start/stop
# mark the first/last write into each bank.
QK_PIECES = [
    (0, 0, 0, 128, True, False),
    (1, 128, 0, 256, False, False),
    (2, 384, 0, 128, False, True),     # last write to bank 0
    (2, 512, 128, 256, True, False),   # first write to bank 1
    (3, 768, 128, 256, False, True),   # last write to bank 1
    (3, 1024, 384, 128, True, False),  # first write to bank 2
    (4, 1152, 256, 256, False, False),
    (5, 1408, 384, 128, False, True),  # last write to bank 2
]

# Chunked tri-mask fallback (only used when opts["mask_full"] is False):
# (score_col, tri_col, width).  The default path multiplies the whole
# [128, 1536] P^T tile by a precomputed full-width mask in one DVE op.
MASKS = [
    (0, 0, 128),
    (256, 0, 256),
    (640, 0, 256),
    (1024, 0, 256),
    (1408, 128, 128),
]

# AV matmuls: (kt, score_col, width, out_q_col).  kt3 ([128,512)) runs first
# with start=True, then kt0 ([0,128)); after those two every output column is
# written, so the remaining matmuls are uniformly accumulating (CoreSim
# asserts uniform fresh-or-accumulate per matmul; this order avoids splits).
AV_TABLE = [
    (3, 768, 384, 128),
    (0, 0, 128, 0),
    (1, 128, 256, 0),
    (2, 384, 384, 0),
    (4, 1152, 256, 256),
    (5, 1408, 128, 384),
]


OPTS = {
    "mask_engine": "vector",   # "vector" | "gpsimd"
    "sc_bufs": 2,              # score-tile double buffering (3 PSUM banks each)
    "ost_engine": "vector",    # (unused when alternating) PSUM->SBUF copy engine
    "out_bf16": True,          # write the partial projection output as bf16
}


def _emit(tc, qT_d, kT_d, vE_d, Wc_d, tri_d, out_d, taps=None, reps=1, opts=None):
    import concourse.mybir as mybir

    o = dict(OPTS)
    if opts:
        o.update(opts)
    nc = tc.nc
    BF16 = mybir.dt.bfloat16
    F32 = mybir.dt.float32
    Exp = mybir.ActivationFunctionType.Exp
    Copy = mybir.ActivationFunctionType.Copy
    mask_mul = (nc.gpsimd.tensor_mul if o["mask_engine"] == "gpsimd"
                else nc.vector.tensor_mul)

    def one_rep(ctx):
        const = ctx.enter_context(tc.tile_pool(name="const", bufs=1))
        pt_pool = ctx.enter_context(tc.tile_pool(name="pt", bufs=4))
        rc_pool = ctx.enter_context(tc.tile_pool(name="rc", bufs=3))
        yt_pool = ctx.enter_context(tc.tile_pool(name="yt", bufs=1))
        ost_pool = ctx.enter_context(tc.tile_pool(name="ost", bufs=3))

        # ---- input DMAs: q/k/v split per head-pair so head 0 compute can
        # start after ~1/4 of the data has landed.  tri is loaded first (the
        # head-0 mask needs it); W last (only the projection needs it).  The
        # ones-columns of vE are memset on the otherwise-idle GPSIMD engine
        # instead of being DMA'd (saves 0.75 MB of HBM traffic per pass).
        tri_t = const.tile([128, 1536], BF16, tag="tri", name="tri")
        nc.sync.dma_start(tri_t[:], tri_d)
        q_hp, k_hp, v_hp = [], [], []
        for hp in range(4):
            qt_ = const.tile([65, 2 * QL], BF16, tag=f"q{hp}", name=f"q{hp}")
            nc.sync.dma_start(qt_[:], qT_d[:, hp * 2 * QL:(hp + 1) * 2 * QL])
            q_hp.append(qt_)
            kt_ = const.tile([65, 2 * KW], BF16, tag=f"k{hp}", name=f"k{hp}")
            nc.sync.dma_start(kt_[:], kT_d[:, hp * 2 * KW:(hp + 1) * 2 * KW])
            k_hp.append(kt_)
            vt_ = const.tile([128, 2 * KW], BF16, tag=f"v{hp}", name=f"v{hp}")
            nc.gpsimd.memset(vt_[:], 1.0)
            nc.sync.dma_start(
                vt_[:].rearrange("p (b c) -> p b c", c=128)[:, :, 64:128],
                vE_d[:, hp * 2 * (KW // 2):(hp + 1) * 2 * (KW // 2)].rearrange(
                    "p (b c) -> p b c", c=64),
            )
            v_hp.append(vt_)

        def q_sl(h, a, b):
            return q_hp[h // 2][:, (h % 2) * QL + a:(h % 2) * QL + b]

        def k_sl(h, a, b):
            return k_hp[h // 2][:, (h % 2) * KW + a:(h % 2) * KW + b]

        def v_sl(h, a, b):
            return v_hp[h // 2][:, (h % 2) * KW + a:(h % 2) * KW + b]

        wt_all = const.tile([128, 4 * DIM], BF16, tag="w", name="w")
        nc.sync.dma_start(wt_all[:], Wc_d)

        yt_t = [
            yt_pool.tile([128, QL], BF16, tag=f"yt{hp}", name=f"yt{hp}")
            for hp in range(4)
        ]

        # ---- attention per head ----
        with ExitStack() as actx:
            sc_pool = actx.enter_context(
                tc.tile_pool(name="sc", bufs=o["sc_bufs"], space="PSUM"))
            ot_pool = actx.enter_context(
                tc.tile_pool(name="ot", bufs=2, space="PSUM"))
            for h in range(HL):
                if o.get("skip_heads"):
                    break
                sc = sc_pool.tile([128, 1536], F32, tag="sc", name="sc")
                for (kt, so, qo, w, st, sp) in QK_PIECES:
                    nc.tensor.matmul(
                        sc[:, so:so + w],
                        lhsT=k_sl(h, kt * 128, (kt + 1) * 128),
                        rhs=q_sl(h, qo, qo + w),
                        start=st, stop=sp,
                    )
                pt = pt_pool.tile([128, 1536], BF16, tag="pt", name="pt")
                if not o.get("skip_exp"):
                    nc.scalar.activation(pt[:], sc[:], Exp, scale=1.0 / math.sqrt(D))
                if not o.get("skip_masks"):
                    if o.get("mask_full", True):
                        mask_mul(pt[:], pt[:], tri_t[:])
                    else:
                        for (off, toff, tw) in MASKS:
                            mask_mul(
                                pt[:, off:off + tw],
                                pt[:, off:off + tw],
                                tri_t[:, toff:toff + tw],
                            )
                ot = ot_pool.tile([128, QL], F32, tag="ot", name="ot")
                if o.get("skip_av"):
                    continue
                for i, (kt, po, w, oq) in enumerate(AV_TABLE):
                    nc.tensor.matmul(
                        ot[:, oq:oq + w],
                        lhsT=v_sl(h, kt * 128, (kt + 1) * 128),
                        rhs=pt[:, po:po + w],
                        start=(i == 0), stop=(i == len(AV_TABLE) - 1),
                    )
                # vE packs [ones | v] per key tile, so ot rows 0:64 hold the
                # softmax denominator (replicated) and rows 64:128 hold y^T.
                # Custom DVE ops only work at base partition 0 on HW; this
                # layout keeps the reciprocal there.
                rc = rc_pool.tile([64, QL], F32, tag="rc", name="rc")
                if not o.get("skip_norm"):
                    nc.vector.reciprocal_approx_fast(out=rc[:], in_=ot[0:64, :])
                    hp, odd = h // 2, h % 2
                    nc.vector.tensor_mul(
                        yt_t[hp][odd * 64:(odd + 1) * 64, :], ot[64:128, :], rc[:]
                    )
                if taps is not None and h == 0:
                    nc.sync.dma_start(taps["pt0"], pt[:])
                    nc.sync.dma_start(taps["rc0"], rc[:])
                    ots = ost_pool.tile([128, QL], F32, tag="ots", name="ots")
                    nc.vector.tensor_copy(ots[:], ot[:])
                    nc.sync.dma_start(taps["ot0"], ots[:])

        if taps is not None:
            for hp in range(4):
                nc.sync.dma_start(taps[f"yt{hp}"], yt_t[hp][:])

        if o.get("skip_proj"):
            return
        # ---- output projection: out[q, n] = sum_hp yT_hp.T @ W_hp ----
        OUT_DT = BF16 if o.get("out_bf16", True) else F32
        with tc.tile_pool(name="proj", bufs=3, space="PSUM") as proj_pool:
            for qt in range(NQB):
                op_t = proj_pool.tile([128, 1024], F32, tag="op", name="op")
                for nh in range(2):
                    for hp in range(4):
                        nc.tensor.matmul(
                            op_t[:, nh * 512:(nh + 1) * 512],
                            lhsT=yt_t[hp][:, qt * 128:(qt + 1) * 128],
                            rhs=wt_all[:, hp * DIM + nh * 512:
                                       hp * DIM + nh * 512 + 512],
                            start=(hp == 0), stop=(hp == 3),
                        )
                ost = ost_pool.tile([128, 1024], OUT_DT, tag="ost", name="ost")
                if qt % 2 == 0:
                    nc.scalar.activation(ost[:], op_t[:], Copy)
                else:
                    nc.vector.tensor_copy(ost[:], op_t[:])
                nc.sync.dma_start(
                    out_d[qt * 128:(qt + 1) * 128, :], ost[:],
                )

    for _rep in range(reps):
        with ExitStack() as ctx:
            one_rep(ctx)


def build_program(debug_taps=False, reps=1, opts=None):
    """Build + compile the SPMD program once.  Returns the Bacc object."""
    from concourse import bacc, tile
    import concourse.mybir as mybir

    BF16 = mybir.dt.bfloat16
    F32 = mybir.dt.float32

    nc = bacc.Bacc("TRN2", target_bir_lowering=False, debug=False, num_devices=8)
    qT_d = nc.dram_tensor("qT", [65, HL * QL], BF16, kind="ExternalInput").ap()
    kT_d = nc.dram_tensor("kT", [65, HL * KW], BF16, kind="ExternalInput").ap()
    vE_d = nc.dram_tensor("vE", [128, HL * (KW // 2)], BF16, kind="ExternalInput").ap()
    Wc_d = nc.dram_tensor("Wc", [128, 4 * DIM], BF16, kind="ExternalInput").ap()
    tri_d = nc.dram_tensor("tri", [128, 1536], BF16, kind="ExternalInput").ap()
    out_dt = BF16 if (opts or {}).get("out_bf16", OPTS.get("out_bf16", True)) else F32
    out_d = nc.dram_tensor("out", [QL, DIM], out_dt, kind="ExternalOutput").ap()

    taps = None
    if debug_taps:
        taps = {
            "pt0": nc.dram_tensor("pt0", [128, 1536], BF16, kind="ExternalOutput").ap(),
            "rc0": nc.dram_tensor("rc0", [64, QL], F32, kind="ExternalOutput").ap(),
            "ot0": nc.dram_tensor("ot0", [128, QL], F32, kind="ExternalOutput").ap(),
        }
        for hp in range(4):
            taps[f"yt{hp}"] = nc.dram_tensor(
                f"yt{hp}", [128, QL], BF16, kind="ExternalOutput"
            ).ap()

    with tile.TileContext(nc) as tc:
        _emit(tc, qT_d, kT_d, vE_d, Wc_d, tri_d, out_d, taps=taps, reps=reps, opts=opts)
    nc.compile()
    return nc


def pack_inputs(q, k, v, W_proj):
    """Shard + lay out the full inputs for the 8 cores.  Returns in_maps."""
    q = np.asarray(q, dtype=np.float32)
    k = np.asarray(k, dtype=np.float32)
    v = np.asarray(v, dtype=np.float32)
    W = np.asarray(W_proj, dtype=np.float32)

    p_idx = np.arange(128)[:, None]
    i_idx = np.arange(128)[None, :]
    lo = (p_idx > i_idx).astype(np.float32)
    hi = (p_idx <= i_idx).astype(np.float32)
    one = np.ones((128, 128), np.float32)
    # full-width [128, 1536] mask matching the score-tile chunk layout
    chunk_masks = [lo, one, lo, hi, one, lo, hi, one, lo, hi, one, hi]
    tri = np.concatenate(chunk_masks, axis=1).astype(BF16NP)

    in_maps = []
    for c in range(8):
        r, s = c // S, c % S
        hs = slice(r * HL, (r + 1) * HL)
        qs = slice(s * QL, (s + 1) * QL)

        qh = q[0, hs, qs, :]                      # (HL, QL, D)
        qT = np.empty((HL, 65, QL), dtype=np.float32)
        qT[:, :64, :] = qh.transpose(0, 2, 1)
        qT[:, 64, :] = 1.0

        j0 = s * QL - WINDOW
        idx = j0 + np.arange(KW)
        valid = idx >= 0
        kh = np.zeros((HL, KW, D), dtype=np.float32)
        vh = np.zeros((HL, KW, D), dtype=np.float32)
        kh[:, valid] = k[0, hs][:, idx[valid], :]
        vh[:, valid] = v[0, hs][:, idx[valid], :]

        kT = np.empty((HL, 65, KW), dtype=np.float32)
        kT[:, :64, :] = kh.transpose(0, 2, 1)
        kT[:, 64, :] = np.where(valid, 0.0, NEG)[None, :]

        vE = np.empty((HL, 128, NKT * 64), dtype=np.float32)
        for kt in range(NKT):
            vE[:, :, kt * 64:(kt + 1) * 64] = vh[:, kt * 128:(kt + 1) * 128, :]

        Wc = np.ascontiguousarray(
            W[r * 512:(r + 1) * 512, :].reshape(4, 128, DIM)
        )

        # batched SBUF layouts: heads concatenated along the free dim
        qT_b = np.ascontiguousarray(qT.transpose(1, 0, 2).reshape(65, HL * QL))
        kT_b = np.ascontiguousarray(kT.transpose(1, 0, 2).reshape(65, HL * KW))
        vE_b = np.ascontiguousarray(
            vE.transpose(1, 0, 2).reshape(128, HL * (KW // 2)))
        Wc_b = np.ascontiguousarray(Wc.transpose(1, 0, 2).reshape(128, 4 * DIM))

        in_maps.append({
            "qT": qT_b.astype(BF16NP),
            "kT": kT_b.astype(BF16NP),
            "vE": vE_b.astype(BF16NP),
            "Wc": Wc_b.astype(BF16NP),
            "tri": tri,
        })
    return in_maps


def combine_outputs(results):
    """results[c]["out"] -> full (B, T, DIM) float32 output."""
    out = np.zeros((B, T, DIM), dtype=np.float32)
    for c in range(8):
        r, s = c // S, c % S
        out[0, s * QL:(s + 1) * QL, :] += np.asarray(
            results[c]["out"], dtype=np.float32)
    return out


_PROGRAM = None


def _get_program():
    global _PROGRAM
    if _PROGRAM is None:
        _PROGRAM = build_program()
    return _PROGRAM


def kernel(q, k, v, W_proj):
    from concourse.bass_utils import run_bass_kernel_spmd

    nc = _get_program()
    in_maps = pack_inputs(q, k, v, W_proj)
    res = run_bass_kernel_spmd(nc, in_maps, list(range(8)))
    return combine_outputs(res.results)


if __name__ == "__main__":
    # smoke test with random data
    rng = np.random.default_rng(0)
    q = rng.standard_normal((B, H, T, D), dtype=np.float32)
    k = rng.standard_normal((B, H, T, D), dtype=np.float32)
    v = rng.standard_normal((B, H, T, D), dtype=np.float32)
    W = rng.standard_normal((DIM, DIM), dtype=np.float32) / math.sqrt(DIM)
    out = kernel(q, k, v, W)
    print(out.shape, out.dtype, np.abs(out).mean())



# revision 17
# speedup vs baseline: 12.8510x; 12.8510x over previous
"""Trainium2 Bass kernel: causal sliding-window attention + output projection.

Reference computation (B=1, H=16, T=2048, D=64, WINDOW=256, DIM=1024):
    att  = softmax(mask(q @ k^T / sqrt(D)))       per head, sliding causal window
    y    = att @ v                                 -> (B, H, T, D)
    out  = y.transpose -> (B, T, H*D) @ W_proj     -> (B, T, DIM)

Sharding over 8 NeuronCores: 2 head-groups (R) x 4 sequence-blocks (S).
Core c = (r, s): heads [8r, 8r+8), queries [512s, 512s+512), key window
[512s-256, 512s+512) (zero-padded below 0, NEG bias row kills padded keys).
W_proj row-sharded per head group; host sums the two partial projections per
sequence block (the "all-reduce after projection" done at gather time).

On-device layout (transposed so no on-chip transposes are needed):
  scores^T[k, q] = (kT_ext)^T @ qT_ext   65-row contraction: 64 dims + bias
                                         row giving -1e9 on padded keys.
  Each head is split into two half-stages (key tiles 0-2 / 3-5); each half is
  a [128, 768] f32 score tile living in a 2-bank PSUM slot so the attention
  PSUM footprint is sc(2x2 banks) + ot(2x1) = 6 banks, leaving 2 banks for
  the projection accumulators.  All tile pools persist across reps: rep N+1
  input DMAs and QK overlap rep N's projection, and there is no per-rep
  drain/barrier.
  P^T = exp(scores / 8)                  one ACT op per half [128, 768]
  3 chunked DVE multiplies per half apply the lo/hi triangle masks from a
  shared [128, 256] tri tile (strided access pattern pairs chunks).
  O = [ones | v]^T @ P^T                 ones|v read from one SBUF tile via a
                                         2-level AP: cols 0:64 are a memset
                                         ones block, v is contiguous after,
                                         so the v DMA lands with 6 KB
                                         contiguous elements per partition.
  rows 0:64 of O = softmax denominator, rows 64:128 = y^T.
  yT = O[64:128] / O[0:64]               one DVE tensor_tensor divide
  out[q, n] = sum_hp yT_hp^T @ W_hp      8 groups of 4 matmuls into 1-bank
                                         [128,512] PSUM tiles; copies to SBUF
                                         alternate ACT/DVE; DMA out per qt.
The PE stream runs one half-stage ahead of the AV consumers (QK of stage
s+1 is issued before AV of stage s) so the tensor engine stays busy and
ramps to its fast p-state.
"""

import math
import os
import sys

import numpy as np

for _p in ("/opt/trn_rl_repo",):
    if _p not in sys.path and os.path.isdir(_p):
        sys.path.insert(0, _p)

import ml_dtypes  # noqa: E402

BF16NP = ml_dtypes.bfloat16

B, H, T, D = 1, 16, 2048, 64
DIM = H * D
WINDOW = 256
R, S = 2, 4                 # head groups x sequence blocks
HL = H // R                 # 8 heads per core
QL = T // S                 # 512 queries per core
KW = QL + WINDOW            # 768-key window per core
NKT = KW // 128             # 6 key tiles
NQB = QL // 128             # 4 query blocks
NEG = -1.0e9                # additive bias for padded (out-of-range) keys
VWT_COLS = 8 * 6 * 128 + 4 * 1024 + 256   # [ v | W | tri ] combined tile

# Half-stage QK pieces: (kt, sc_col, q_col, width, start, stop).
# Score halves live in [128, 768] f32 inside a 2-bank (1024-col) PSUM slot;
# pieces never cross the 512-col bank line; start/stop mark first/last
# write per bank.
QK_A = [
    (0, 0, 0, 128, True, False),
    (1, 128, 0, 256, False, False),
    (2, 384, 0, 128, False, True),     # bank 0 done
    (2, 512, 128, 256, True, True),    # bank 1
]
QK_B = [
    (3, 0, 128, 256, True, False),
    (3, 256, 384, 128, False, False),
    (4, 384, 256, 128, False, True),   # bank 0 done
    (4, 512, 384, 128, True, False),   # bank 1
    (5, 640, 384, 128, False, True),
]

# AV pieces per half: (kt, pt_col, width, out_q_col).  Emission order within
# the whole head: A first (kt2 opens the ot accumulation group covering
# q0:384), B's kt5 is the only other fresh-write (q384:512); everything else
# accumulates, so the fresh-or-accumulate groups stay uniform per matmul.
AV_A = [
    (2, 384, 384, 0),       # start=True (opens group, q 0:384)
    (0, 0, 128, 0),
    (1, 128, 256, 0),
]
AV_B = [
    (5, 640, 128, 384),     # fresh write q384:512
    (3, 0, 384, 128),
    (4, 384, 256, 256),
]

# Mask ops per half: (pt_col, n_chunks, tri_col, engine).  Each op multiplies
# n_chunks 128-col chunks of pt (stride 128) by tri chunks starting at
# tri_col with stride 128.  tri = [lo | hi] (2 x 128 cols).  The 128-col ops
# go to the otherwise-idle GPSIMD; the 2-chunk op (which gates the
# group-opening AV matmul) stays on the faster DVE.
# half A chunks: [lo, one, lo, hi, one, lo] -> mask c0(lo), c2+c3(lo,hi), c5(lo)
# half B chunks: [hi, one, lo, hi, one, hi] -> mask c0(hi), c2+c3(lo,hi), c5(hi)
MASKS_A = [(256, 2, 0, "v"), (640, 1, 0, "g"), (0, 1, 0, "g")]
MASKS_B = [(256, 2, 0, "v"), (640, 1, 128, "g"), (0, 1, 128, "g")]

OPTS = {
    # one DVE divide would be ideal but the BIR verifier forbids two PSUM
    # inputs on TensorTensor, so default to recip+mul (one PSUM input each)
    "div_dve": False,
    "lookahead": 1,       # PE half-stages of QK lookahead before AV
    "pool_masks": True,   # run 128-col mask ops on GPSIMD instead of DVE
    "pipeline_proj": True,  # interleave proj of rep N-1 into rep N attention
    "op2_pool": 2,        # 0/1/2: fraction of 2-chunk mask ops on GPSIMD
    "ncopies_act": 2,     # of the 8 proj copies, how many go to ACT
}


def _chunk_ap(AP, t, col, n):
    """[128, n, 128] AP over n 128-col chunks of t starting at col."""
    base = t[:, col:col + 128]
    return AP(base.tensor, base.offset,
              [list(base.ap[0]), [128, n], [1, 128]])


def _emit(tc, qkT_d, vwt_d, out_d, reps=1, opts=None):
    import concourse.mybir as mybir
    from concourse.bass import AP

    o = dict(OPTS)
    if opts:
        o.update(opts)
    nc = tc.nc
    BF16 = mybir.dt.bfloat16
    F32 = mybir.dt.float32
    Exp = mybir.ActivationFunctionType.Exp
    Copy = mybir.ActivationFunctionType.Copy
    Div = mybir.AluOpType.divide

    # ---- persistent pools (cross-rep pipelining, no per-rep barriers) ----
    from contextlib import ExitStack
    ctx = ExitStack()
    qk_pool = ctx.enter_context(tc.tile_pool(name="qk", bufs=2))
    v_pool = ctx.enter_context(tc.tile_pool(name="v", bufs=3))
    pt_pool = ctx.enter_context(tc.tile_pool(name="pt", bufs=4))
    yt_pool = ctx.enter_context(tc.tile_pool(name="yt", bufs=2))
    ost_pool = ctx.enter_context(tc.tile_pool(name="ost", bufs=2))
    sc_pool = ctx.enter_context(tc.tile_pool(name="sc", bufs=2, space="PSUM"))
    ot_pool = ctx.enter_context(tc.tile_pool(name="ot", bufs=2, space="PSUM"))
    op_pool = ctx.enter_context(tc.tile_pool(name="op", bufs=2, space="PSUM"))

    def emit_proj_group(g, yt_t, wt, ost_state):
        """Proj group g (qt = g//2, nh = g%2): 4 matmuls + copy; DMA per qt
        pair.  ost_state carries the current [128, 2048] staging tile."""
        qt, nh = g // 2, g % 2
        if g % 4 == 0:
            ost_state[0] = ost_pool.tile([128, 2048], BF16, tag="ost", name="ost")
        ost = ost_state[0]
        op_t = op_pool.tile([128, 512], F32, tag="op", name="op")
        for hp in range(4):
            nc.tensor.matmul(
                op_t[:],
                lhsT=yt_t[hp][:, qt * 128:(qt + 1) * 128],
                rhs=wt[:, hp * DIM + nh * 512:hp * DIM + nh * 512 + 512],
                start=(hp == 0), stop=(hp == 3),
            )
        dst = ost[:, (g % 4) * 512:(g % 4 + 1) * 512]
        if g % 8 < o["ncopies_act"]:
            nc.scalar.activation(dst, op_t[:], Copy)
        else:
            nc.vector.tensor_copy(dst, op_t[:])
        if g % 4 == 3:
            pair = g // 4
            src = AP(ost.tensor, ost.offset,
                     [list(ost.ap[0]), [1024, 2], [1, 1024]])
            dstd = AP(out_d.tensor, pair * 256 * DIM,
                      [[DIM, 128], [128 * DIM, 2], [1, DIM]])
            nc.sync.dma_start(dstd, src)

    def emit_inputs():
        """Allocate + DMA one rep's inputs.  Called one rep ahead so the
        transfers land before the consuming rep starts."""
        qk_t = qk_pool.tile([65, 4 * 2560], BF16, tag="qk", name="qk")
        nc.sync.dma_start(qk_t[:], qkT_d)
        # vwt holds [ v | W | tri ] in one DMA: v as [ones(64)|v(64)] per
        # (head, key-tile) interleaved in DRAM at pack time, so each AV lhsT
        # is a plain [128, 128] slice and the transfer is fully contiguous.
        vwt = v_pool.tile([128, VWT_COLS], BF16, tag="vwt", name="vwt")
        nc.sync.dma_start(vwt[:], vwt_d)
        return qk_t, vwt

    def one_rep(inputs, prev):
        qk_t, vwt = inputs
        vall = vwt[:, 0:8 * NKT * 128]
        wt = vwt[:, 8 * NKT * 128:8 * NKT * 128 + 4 * DIM]
        tri_t = vwt[:, 8 * NKT * 128 + 4 * DIM:VWT_COLS]

        def q_sl(h, a, b):
            base = (h // 2) * 2560 + (h % 2) * QL
            return qk_t[:, base + a:base + b]

        def k_sl(h, a, b):
            base = (h // 2) * 2560 + 2 * QL + (h % 2) * KW
            return qk_t[:, base + a:base + b]

        def v_lhsT(h, kt):
            b = (h * NKT + kt) * 128
            return vall[:, b:b + 128]

        yt_t = [
            yt_pool.tile([128, QL], BF16, tag=f"yt{hp}", name=f"yt{hp}")
            for hp in range(4)
        ]

        # ---- attention: 16 half-stages, PE runs `lookahead` ahead;
        # proj groups of the previous rep interleave every 2nd stage ----
        stages = [(h, half) for h in range(HL) for half in range(2)]
        pend = []          # (h, half, pt) awaiting AV
        ot_cur = [None]
        ost_state = [None]

        def emit_qk(h, half):
            sc = sc_pool.tile([128, 1024], F32, tag="sc", name="sc")
            table = QK_A if half == 0 else QK_B
            for (kt, so, qo, w, st, sp) in table:
                nc.tensor.matmul(
                    sc[:, so:so + w],
                    lhsT=k_sl(h, kt * 128, (kt + 1) * 128),
                    rhs=q_sl(h, qo, qo + w),
                    start=st, stop=sp,
                )
            pt = pt_pool.tile([128, 768], BF16, tag="pt", name="pt")
            nc.scalar.activation(pt[:], sc[:, 0:768], Exp, scale=1.0 / math.sqrt(D))
            masks = MASKS_A if half == 0 else MASKS_B
            stage_idx = h * 2 + half
            for (pc, nck, tc_, eng) in masks:
                use_pool = (eng == "g" and o["pool_masks"]) or (
                    eng == "v" and stage_idx % 2 < o["op2_pool"])
                mul = nc.gpsimd.tensor_mul if use_pool else nc.vector.tensor_mul
                mul(
                    _chunk_ap(AP, pt, pc, nck),
                    _chunk_ap(AP, pt, pc, nck),
                    _chunk_ap(AP, tri_t, tc_, nck) if nck > 1
                    else tri_t[:, tc_:tc_ + 128].unsqueeze(1),
                )
            return pt

        def emit_av(h, half, pt):
            if half == 0:
                ot_cur[0] = ot_pool.tile([128, QL], F32, tag="ot", name="ot")
            ot = ot_cur[0]
            table = AV_A if half == 0 else AV_B
            for i, (kt, po, w, oq) in enumerate(table):
                nc.tensor.matmul(
                    ot[:, oq:oq + w],
                    lhsT=v_lhsT(h, kt),
                    rhs=pt[:, po:po + w],
                    start=(half == 0 and i == 0),
                    stop=(half == 1 and i == len(table) - 1),
                )
            if half == 1:
                hp, odd = h // 2, h % 2
                dst = yt_t[hp][odd * 64:(odd + 1) * 64, :]
                if o["div_dve"]:
                    nc.vector.tensor_tensor(
                        out=dst, in0=ot[64:128, :], in1=ot[0:64, :], op=Div)
                else:
                    rc = pt_pool.tile([64, QL], F32, tag="rc", name="rc")
                    nc.vector.reciprocal_approx_fast(out=rc[:], in_=ot[0:64, :])
                    nc.vector.tensor_mul(dst, ot[64:128, :], rc[:])

        la = o["lookahead"]
        for i, (h, half) in enumerate(stages):
            pend.append((h, half, emit_qk(h, half)))
            while len(pend) > la:
                ph, phalf, ppt = pend.pop(0)
                emit_av(ph, phalf, ppt)
            if prev is not None and i % 2 == 1:
                emit_proj_group(i // 2, prev[0], prev[1], ost_state)
        for (ph, phalf, ppt) in pend:
            emit_av(ph, phalf, ppt)

        return yt_t, wt

    prev = None
    pipeline = o["pipeline_proj"]
    cur_in = emit_inputs()
    for _rep in range(reps):
        nxt_in = emit_inputs() if _rep + 1 < reps else None
        cur = one_rep(cur_in, prev if pipeline else None)
        cur_in = nxt_in
        if pipeline:
            prev = cur
        else:
            ost_state = [None]
            for g in range(8):
                emit_proj_group(g, cur[0], cur[1], ost_state)
    if pipeline and prev is not None:
        ost_state = [None]
        for g in range(8):
            emit_proj_group(g, prev[0], prev[1], ost_state)
    ctx.close()


def build_program(reps=1, opts=None):
    """Build + compile the SPMD program once.  Returns the Bacc object."""
    from concourse import bacc, tile
    import concourse.mybir as mybir

    BF16 = mybir.dt.bfloat16

    nc = bacc.Bacc("TRN2", target_bir_lowering=False, debug=False, num_devices=8)
    qkT_d = nc.dram_tensor("qkT", [65, 4 * 2560], BF16, kind="ExternalInput").ap()
    vwt_d = nc.dram_tensor("vwt", [128, VWT_COLS], BF16, kind="ExternalInput").ap()
    out_d = nc.dram_tensor("out", [QL, DIM], BF16, kind="ExternalOutput").ap()

    with tile.TileContext(nc) as tc:
        _emit(tc, qkT_d, vwt_d, out_d, reps=reps, opts=opts)
    nc.compile()
    return nc


def pack_inputs(q, k, v, W_proj):
    """Shard + lay out the full inputs for the 8 cores.  Returns in_maps."""
    q = np.asarray(q, dtype=np.float32)
    k = np.asarray(k, dtype=np.float32)
    v = np.asarray(v, dtype=np.float32)
    W = np.asarray(W_proj, dtype=np.float32)

    p_idx = np.arange(128)[:, None]
    i_idx = np.arange(128)[None, :]
    lo = (p_idx > i_idx).astype(np.float32)
    hi = (p_idx <= i_idx).astype(np.float32)
    tri_f32 = np.concatenate([lo, hi], axis=1)

    in_maps = []
    for c in range(8):
        r, s = c // S, c % S
        hs = slice(r * HL, (r + 1) * HL)
        qs = slice(s * QL, (s + 1) * QL)

        qh = q[0, hs, qs, :]                      # (HL, QL, D)
        qT = np.empty((HL, 65, QL), dtype=np.float32)
        qT[:, :64, :] = qh.transpose(0, 2, 1)
        qT[:, 64, :] = 1.0

        j0 = s * QL - WINDOW
        idx = j0 + np.arange(KW)
        valid = idx >= 0
        kh = np.zeros((HL, KW, D), dtype=np.float32)
        vh = np.zeros((HL, KW, D), dtype=np.float32)
        kh[:, valid] = k[0, hs][:, idx[valid], :]
        vh[:, valid] = v[0, hs][:, idx[valid], :]

        kT = np.empty((HL, 65, KW), dtype=np.float32)
        kT[:, :64, :] = kh.transpose(0, 2, 1)
        kT[:, 64, :] = np.where(valid, 0.0, NEG)[None, :]

        # qkT: per head-pair block [q h0 | q h1 | k h0 | k h1]
        qkT = np.empty((65, 4 * 2560), dtype=np.float32)
        for hp in range(4):
            b = hp * 2560
            qkT[:, b:b + QL] = qT[2 * hp]
            qkT[:, b + QL:b + 2 * QL] = qT[2 * hp + 1]
            qkT[:, b + 2 * QL:b + 2 * QL + KW] = kT[2 * hp]
            qkT[:, b + 2 * QL + KW:b + 2560] = kT[2 * hp + 1]

        # vE: per (head, key-tile) a [128, 128] block = [ones(64) | v(64)]
        vE = np.empty((128, 8 * NKT * 128), dtype=np.float32)
        for h in range(HL):
            for kt in range(NKT):
                b = (h * NKT + kt) * 128
                vE[:, b:b + 64] = 1.0
                vE[:, b + 64:b + 128] = vh[h, kt * 128:(kt + 1) * 128, :]

        Wc = np.ascontiguousarray(
            W[r * 512:(r + 1) * 512, :].reshape(4, 128, DIM)
        )
        Wc_b = np.ascontiguousarray(Wc.transpose(1, 0, 2).reshape(128, 4 * DIM))

        vwt = np.concatenate([vE, Wc_b, tri_f32], axis=1)

        in_maps.append({
            "qkT": qkT.astype(BF16NP),
            "vwt": vwt.astype(BF16NP),
        })
    return in_maps


def combine_outputs(results):
    """results[c]["out"] -> full (B, T, DIM) float32 output."""
    out = np.zeros((B, T, DIM), dtype=np.float32)
    for c in range(8):
        r, s = c // S, c % S
        out[0, s * QL:(s + 1) * QL, :] += np.asarray(
            results[c]["out"], dtype=np.float32)
    return out


_PROGRAM = None


def _get_program():
    global _PROGRAM
    if _PROGRAM is None:
        _PROGRAM = build_program()
    return _PROGRAM


def kernel(q, k, v, W_proj):
    from concourse.bass_utils import run_bass_kernel_spmd

    nc = _get_program()
    in_maps = pack_inputs(q, k, v, W_proj)
    res = run_bass_kernel_spmd(nc, in_maps, list(range(8)))
    return combine_outputs(res.results)


if __name__ == "__main__":
    # smoke test with random data
    rng = np.random.default_rng(0)
    q = rng.standard_normal((B, H, T, D), dtype=np.float32)
    k = rng.standard_normal((B, H, T, D), dtype=np.float32)
    v = rng.standard_normal((B, H, T, D), dtype=np.float32)
    W = rng.standard_normal((DIM, DIM), dtype=np.float32) / math.sqrt(DIM)
    out = kernel(q, k, v, W)
    print(out.shape, out.dtype, np.abs(out).mean())


# revision 20
# speedup vs baseline: 14.9972x; 1.1670x over previous
"""Trainium2 Bass kernel: causal sliding-window attention + output projection.

Reference computation (B=1, H=16, T=2048, D=64, WINDOW=256, DIM=1024):
    att  = softmax(mask(q @ k^T / sqrt(D)))       per head, sliding causal window
    y    = att @ v                                 -> (B, H, T, D)
    out  = y.transpose -> (B, T, H*D) @ W_proj     -> (B, T, DIM)

Sharding over 8 NeuronCores: 2 head-groups (R) x 4 sequence-blocks (S).
Core c = (r, s): heads [8r, 8r+8), queries [512s, 512s+512), key window
[512s-256, 512s+512) (zero-padded below 0, NEG bias row kills padded keys).
W_proj row-sharded per head group; host sums the two partial projections per
sequence block (the "all-reduce after projection" done at gather time).

On-device layout (transposed so no on-chip transposes are needed):
  scores^T[k, q] = (kT_ext)^T @ qT_ext   65-row contraction: 64 dims + bias
                                         row giving -1e9 on padded keys.
  Each head is split into two half-stages (key tiles 0-2 / 3-5); each half is
  a [128, 768] f32 score tile living in a 2-bank PSUM slot so the attention
  PSUM footprint is sc(2x2 banks) + ot(2x1) = 6 banks, leaving 2 banks for
  the projection accumulators.  All tile pools persist across reps: rep N+1
  input DMAs and QK overlap rep N's projection, and there is no per-rep
  drain/barrier.
  P^T = exp(scores / 8)                  one ACT op per half [128, 768]
  3 chunked DVE multiplies per half apply the lo/hi triangle masks from a
  shared [128, 256] tri tile (strided access pattern pairs chunks).
  O = [ones | v]^T @ P^T                 ones|v read from one SBUF tile via a
                                         2-level AP: cols 0:64 are a memset
                                         ones block, v is contiguous after,
                                         so the v DMA lands with 6 KB
                                         contiguous elements per partition.
  rows 0:64 of O = softmax denominator, rows 64:128 = y^T.
  yT = O[64:128] / O[0:64]               one DVE tensor_tensor divide
  out[q, n] = sum_hp yT_hp^T @ W_hp      8 groups of 4 matmuls into 1-bank
                                         [128,512] PSUM tiles; copies to SBUF
                                         alternate ACT/DVE; DMA out per qt.
The PE stream runs one half-stage ahead of the AV consumers (QK of stage
s+1 is issued before AV of stage s) so the tensor engine stays busy and
ramps to its fast p-state.
"""

import math
import os
import sys

import numpy as np

for _p in ("/opt/trn_rl_repo",):
    if _p not in sys.path and os.path.isdir(_p):
        sys.path.insert(0, _p)

import ml_dtypes  # noqa: E402

BF16NP = ml_dtypes.bfloat16

B, H, T, D = 1, 16, 2048, 64
DIM = H * D
WINDOW = 256
R, S = 2, 4                 # head groups x sequence blocks
HL = H // R                 # 8 heads per core
QL = T // S                 # 512 queries per core
KW = QL + WINDOW            # 768-key window per core
NKT = KW // 128             # 6 key tiles
NQB = QL // 128             # 4 query blocks
NEG = -1.0e9                # additive bias for padded (out-of-range) keys
VWT_COLS = 8 * 6 * 128 + 4 * 1024 + 1536  # [ v | W | triA | triB ] combined

# Half-stage QK pieces: (kt, sc_col, q_col, width, start, stop).
# Score halves live in [128, 768] f32 inside a 2-bank (1024-col) PSUM slot;
# pieces never cross the 512-col bank line; start/stop mark first/last
# write per bank.
QK_A = [
    (0, 0, 0, 128, True, False),
    (1, 128, 0, 256, False, False),
    (2, 384, 0, 128, False, True),     # bank 0 done
    (2, 512, 128, 256, True, True),    # bank 1
]
QK_B = [
    (3, 0, 128, 256, True, False),
    (3, 256, 384, 128, False, False),
    (4, 384, 256, 128, False, True),   # bank 0 done
    (4, 512, 384, 128, True, False),   # bank 1
    (5, 640, 384, 128, False, True),
]

# AV pieces per half: (kt, pt_col, width, out_q_col).  Emission order within
# the whole head: A first (kt2 opens the ot accumulation group covering
# q0:384), B's kt5 is the only other fresh-write (q384:512); everything else
# accumulates, so the fresh-or-accumulate groups stay uniform per matmul.
AV_A = [
    (2, 384, 384, 0),       # start=True (opens group, q 0:384)
    (0, 0, 128, 0),
    (1, 128, 256, 0),
]
AV_B = [
    (5, 640, 128, 384),     # fresh write q384:512
    (3, 0, 384, 128),
    (4, 384, 256, 256),
]

# Mask ops per half: (pt_col, n_chunks, tri_col, engine).  Each op multiplies
# n_chunks 128-col chunks of pt (stride 128) by tri chunks starting at
# tri_col with stride 128.  tri = [lo | hi] (2 x 128 cols).  The 128-col ops
# go to the otherwise-idle GPSIMD; the 2-chunk op (which gates the
# group-opening AV matmul) stays on the faster DVE.
# half A chunks: [lo, one, lo, hi, one, lo] -> mask c0(lo), c2+c3(lo,hi), c5(lo)
# half B chunks: [hi, one, lo, hi, one, hi] -> mask c0(hi), c2+c3(lo,hi), c5(hi)
MASKS_A = [(256, 2, 0, "v"), (640, 1, 0, "g"), (0, 1, 0, "g")]
MASKS_B = [(256, 2, 0, "v"), (640, 1, 128, "g"), (0, 1, 128, "g")]

OPTS = {
    # one DVE divide would be ideal but the BIR verifier forbids two PSUM
    # inputs on TensorTensor, so default to recip+mul (one PSUM input each)
    "div_dve": False,
    "lookahead": 1,       # PE half-stages of QK lookahead before AV
    "pipeline_proj": True,  # interleave proj of rep N-1 into rep N attention
    "ncopies_act": 4,     # of the 8 proj copies, how many go to ACT
    "mask_full": True,    # one full-width mask op per half (else 3 chunked)
}


def _chunk_ap(AP, t, col, n):
    """[128, n, 128] AP over n 128-col chunks of t starting at col."""
    base = t[:, col:col + 128]
    return AP(base.tensor, base.offset,
              [list(base.ap[0]), [128, n], [1, 128]])


def _emit(tc, qkT_d, vwt_d, out_d, reps=1, opts=None):
    import concourse.mybir as mybir
    from concourse.bass import AP

    o = dict(OPTS)
    if opts:
        o.update(opts)
    nc = tc.nc
    BF16 = mybir.dt.bfloat16
    F32 = mybir.dt.float32
    Exp = mybir.ActivationFunctionType.Exp
    Copy = mybir.ActivationFunctionType.Copy
    Div = mybir.AluOpType.divide

    # ---- persistent pools (cross-rep pipelining, no per-rep barriers) ----
    from contextlib import ExitStack
    ctx = ExitStack()
    qk_pool = ctx.enter_context(tc.tile_pool(name="qk", bufs=2))
    v_pool = ctx.enter_context(tc.tile_pool(name="v", bufs=3))
    pt_pool = ctx.enter_context(tc.tile_pool(name="pt", bufs=4))
    yt_pool = ctx.enter_context(tc.tile_pool(name="yt", bufs=2))
    ost_pool = ctx.enter_context(tc.tile_pool(name="ost", bufs=2))
    sc_pool = ctx.enter_context(tc.tile_pool(name="sc", bufs=2, space="PSUM"))
    ot_pool = ctx.enter_context(tc.tile_pool(name="ot", bufs=2, space="PSUM"))
    op_pool = ctx.enter_context(tc.tile_pool(name="op", bufs=2, space="PSUM"))

    def emit_proj_group(g, yt_t, wt, ost_state):
        """Proj group g (qt = g//2, nh = g%2): 4 matmuls + copy; DMA per qt
        pair.  ost_state carries the current [128, 2048] staging tile."""
        qt, nh = g // 2, g % 2
        if g % 4 == 0:
            ost_state[0] = ost_pool.tile([128, 2048], BF16, tag="ost", name="ost")
        ost = ost_state[0]
        op_t = op_pool.tile([128, 512], F32, tag="op", name="op")
        for hp in range(4):
            nc.tensor.matmul(
                op_t[:],
                lhsT=yt_t[hp][:, qt * 128:(qt + 1) * 128],
                rhs=wt[:, hp * DIM + nh * 512:hp * DIM + nh * 512 + 512],
                start=(hp == 0), stop=(hp == 3),
            )
        dst = ost[:, (g % 4) * 512:(g % 4 + 1) * 512]
        if g % 8 < o["ncopies_act"]:
            nc.scalar.activation(dst, op_t[:], Copy)
        else:
            nc.vector.tensor_copy(dst, op_t[:])
        if g % 4 == 3:
            pair = g // 4
            src = AP(ost.tensor, ost.offset,
                     [list(ost.ap[0]), [1024, 2], [1, 1024]])
            dstd = AP(out_d.tensor, pair * 256 * DIM,
                      [[DIM, 128], [128 * DIM, 2], [1, DIM]])
            nc.sync.dma_start(dstd, src)

    def emit_inputs():
        """Allocate + DMA one rep's inputs.  Called one rep ahead so the
        transfers land before the consuming rep starts."""
        qk_t = qk_pool.tile([65, 4 * 2560], BF16, tag="qk", name="qk")
        nc.sync.dma_start(qk_t[:], qkT_d)
        # vwt holds [ v | W | tri ] in one DMA: v as [ones(64)|v(64)] per
        # (head, key-tile) interleaved in DRAM at pack time, so each AV lhsT
        # is a plain [128, 128] slice and the transfer is fully contiguous.
        vwt = v_pool.tile([128, VWT_COLS], BF16, tag="vwt", name="vwt")
        nc.sync.dma_start(vwt[:], vwt_d)
        return qk_t, vwt

    def one_rep(inputs, prev):
        qk_t, vwt = inputs
        vall = vwt[:, 0:8 * NKT * 128]
        wt = vwt[:, 8 * NKT * 128:8 * NKT * 128 + 4 * DIM]
        trib = 8 * NKT * 128 + 4 * DIM
        tri_half = [vwt[:, trib:trib + 768], vwt[:, trib + 768:trib + 1536]]

        def q_sl(h, a, b):
            base = (h // 2) * 2560 + (h % 2) * QL
            return qk_t[:, base + a:base + b]

        def k_sl(h, a, b):
            base = (h // 2) * 2560 + 2 * QL + (h % 2) * KW
            return qk_t[:, base + a:base + b]

        def v_lhsT(h, kt):
            b = (h * NKT + kt) * 128
            return vall[:, b:b + 128]

        yt_t = [
            yt_pool.tile([128, QL], BF16, tag=f"yt{hp}", name=f"yt{hp}")
            for hp in range(4)
        ]

        # ---- attention: 16 half-stages, PE runs `lookahead` ahead;
        # proj groups of the previous rep interleave every 2nd stage ----
        stages = [(h, half) for h in range(HL) for half in range(2)]
        pend = []          # (h, half, pt) awaiting AV
        ot_cur = [None]
        ost_state = [None]

        def emit_qk(h, half):
            sc = sc_pool.tile([128, 1024], F32, tag="sc", name="sc")
            table = QK_A if half == 0 else QK_B
            for (kt, so, qo, w, st, sp) in table:
                nc.tensor.matmul(
                    sc[:, so:so + w],
                    lhsT=k_sl(h, kt * 128, (kt + 1) * 128),
                    rhs=q_sl(h, qo, qo + w),
                    start=st, stop=sp,
                )
            pt = pt_pool.tile([128, 768], BF16, tag="pt", name="pt")
            nc.scalar.activation(pt[:], sc[:, 0:768], Exp, scale=1.0 / math.sqrt(D))
            if o["mask_full"]:
                # one full-width multiply per half; tri_half holds the
                # precomputed per-half [128, 768] lo/hi/one chunk pattern
                nc.vector.tensor_mul(pt[:], pt[:], tri_half[half])
            else:
                # chunked: skip the all-ones chunks (c1, c4); three DVE ops
                # of 128/256/128 cols against the packed lo|hi tri halves
                tri0 = 8 * NKT * 128 + 4 * DIM + half * 768
                for (pc, nck, tc_) in ((0, 1, 0), (256, 2, 256), (640, 1, 640)):
                    nc.vector.tensor_mul(
                        _chunk_ap(AP, pt, pc, nck),
                        _chunk_ap(AP, pt, pc, nck),
                        _chunk_ap(AP, vwt[:, tri0:tri0 + 768], tc_, nck),
                    )
            return pt

        def emit_av(h, half, pt):
            if half == 0:
                ot_cur[0] = ot_pool.tile([128, QL], F32, tag="ot", name="ot")
            ot = ot_cur[0]
            table = AV_A if half == 0 else AV_B
            for i, (kt, po, w, oq) in enumerate(table):
                nc.tensor.matmul(
                    ot[:, oq:oq + w],
                    lhsT=v_lhsT(h, kt),
                    rhs=pt[:, po:po + w],
                    start=(half == 0 and i == 0),
                    stop=(half == 1 and i == len(table) - 1),
                )
            if half == 1:
                hp, odd = h // 2, h % 2
                dst = yt_t[hp][odd * 64:(odd + 1) * 64, :]
                if o["div_dve"]:
                    nc.vector.tensor_tensor(
                        out=dst, in0=ot[64:128, :], in1=ot[0:64, :], op=Div)
                else:
                    rc = pt_pool.tile([64, QL], F32, tag="rc", name="rc")
                    nc.vector.reciprocal_approx_fast(out=rc[:], in_=ot[0:64, :])
                    nc.vector.tensor_mul(dst, ot[64:128, :], rc[:])

        la = o["lookahead"]
        for i, (h, half) in enumerate(stages):
            pend.append((h, half, emit_qk(h, half)))
            while len(pend) > la:
                ph, phalf, ppt = pend.pop(0)
                emit_av(ph, phalf, ppt)
            if prev is not None and i % 2 == 1:
                emit_proj_group(i // 2, prev[0], prev[1], ost_state)
        for (ph, phalf, ppt) in pend:
            emit_av(ph, phalf, ppt)

        return yt_t, wt

    prev = None
    pipeline = o["pipeline_proj"]
    cur_in = emit_inputs()
    for _rep in range(reps):
        nxt_in = emit_inputs() if _rep + 1 < reps else None
        cur = one_rep(cur_in, prev if pipeline else None)
        cur_in = nxt_in
        if pipeline:
            prev = cur
        else:
            ost_state = [None]
            for g in range(8):
                emit_proj_group(g, cur[0], cur[1], ost_state)
    if pipeline and prev is not None:
        ost_state = [None]
        for g in range(8):
            emit_proj_group(g, prev[0], prev[1], ost_state)
    ctx.close()


def build_program(reps=1, opts=None):
    """Build + compile the SPMD program once.  Returns the Bacc object."""
    from concourse import bacc, tile
    import concourse.mybir as mybir

    BF16 = mybir.dt.bfloat16

    nc = bacc.Bacc("TRN2", target_bir_lowering=False, debug=False, num_devices=8)
    qkT_d = nc.dram_tensor("qkT", [65, 4 * 2560], BF16, kind="ExternalInput").ap()
    vwt_d = nc.dram_tensor("vwt", [128, VWT_COLS], BF16, kind="ExternalInput").ap()
    out_d = nc.dram_tensor("out", [QL, DIM], BF16, kind="ExternalOutput").ap()

    with tile.TileContext(nc) as tc:
        _emit(tc, qkT_d, vwt_d, out_d, reps=reps, opts=opts)
    nc.compile()
    return nc


def pack_inputs(q, k, v, W_proj):
    """Shard + lay out the full inputs for the 8 cores.  Returns in_maps."""
    q = np.asarray(q, dtype=np.float32)
    k = np.asarray(k, dtype=np.float32)
    v = np.asarray(v, dtype=np.float32)
    W = np.asarray(W_proj, dtype=np.float32)

    p_idx = np.arange(128)[:, None]
    i_idx = np.arange(128)[None, :]
    lo = (p_idx > i_idx).astype(np.float32)
    hi = (p_idx <= i_idx).astype(np.float32)
    one = np.ones((128, 128), np.float32)
    triA = np.concatenate([lo, one, lo, hi, one, lo], axis=1)
    triB = np.concatenate([hi, one, lo, hi, one, hi], axis=1)
    tri_f32 = np.concatenate([triA, triB], axis=1)

    in_maps = []
    for c in range(8):
        r, s = c // S, c % S
        hs = slice(r * HL, (r + 1) * HL)
        qs = slice(s * QL, (s + 1) * QL)

        qh = q[0, hs, qs, :]                      # (HL, QL, D)
        qT = np.empty((HL, 65, QL), dtype=np.float32)
        qT[:, :64, :] = qh.transpose(0, 2, 1)
        qT[:, 64, :] = 1.0

        j0 = s * QL - WINDOW
        idx = j0 + np.arange(KW)
        valid = idx >= 0
        kh = np.zeros((HL, KW, D), dtype=np.float32)
        vh = np.zeros((HL, KW, D), dtype=np.float32)
        kh[:, valid] = k[0, hs][:, idx[valid], :]
        vh[:, valid] = v[0, hs][:, idx[valid], :]

        kT = np.empty((HL, 65, KW), dtype=np.float32)
        kT[:, :64, :] = kh.transpose(0, 2, 1)
        kT[:, 64, :] = np.where(valid, 0.0, NEG)[None, :]

        # qkT: per head-pair block [q h0 | q h1 | k h0 | k h1]
        qkT = np.empty((65, 4 * 2560), dtype=np.float32)
        for hp in range(4):
            b = hp * 2560
            qkT[:, b:b + QL] = qT[2 * hp]
            qkT[:, b + QL:b + 2 * QL] = qT[2 * hp + 1]
            qkT[:, b + 2 * QL:b + 2 * QL + KW] = kT[2 * hp]
            qkT[:, b + 2 * QL + KW:b + 2560] = kT[2 * hp + 1]

        # vE: per (head, key-tile) a [128, 128] block = [ones(64) | v(64)]
        vE = np.empty((128, 8 * NKT * 128), dtype=np.float32)
        for h in range(HL):
            for kt in range(NKT):
                b = (h * NKT + kt) * 128
                vE[:, b:b + 64] = 1.0
                vE[:, b + 64:b + 128] = vh[h, kt * 128:(kt + 1) * 128, :]

        Wc = np.ascontiguousarray(
            W[r * 512:(r + 1) * 512, :].reshape(4, 128, DIM)
        )
        Wc_b = np.ascontiguousarray(Wc.transpose(1, 0, 2).reshape(128, 4 * DIM))

        vwt = np.concatenate([vE, Wc_b, tri_f32], axis=1)

        in_maps.append({
            "qkT": qkT.astype(BF16NP),
            "vwt": vwt.astype(BF16NP),
        })
    return in_maps


def combine_outputs(results):
    """results[c]["out"] -> full (B, T, DIM) float32 output."""
    out = np.zeros((B, T, DIM), dtype=np.float32)
    for c in range(8):
        r, s = c // S, c % S
        out[0, s * QL:(s + 1) * QL, :] += np.asarray(
            results[c]["out"], dtype=np.float32)
    return out


_PROGRAM = None


def _get_program():
    global _PROGRAM
    if _PROGRAM is None:
        _PROGRAM = build_program()
    return _PROGRAM


def kernel(q, k, v, W_proj):
    from concourse.bass_utils import run_bass_kernel_spmd

    nc = _get_program()
    in_maps = pack_inputs(q, k, v, W_proj)
    res = run_bass_kernel_spmd(nc, in_maps, list(range(8)))
    return combine_outputs(res.results)


if __name__ == "__main__":
    # smoke test with random data
    rng = np.random.default_rng(0)
    q = rng.standard_normal((B, H, T, D), dtype=np.float32)
    k = rng.standard_normal((B, H, T, D), dtype=np.float32)
    v = rng.standard_normal((B, H, T, D), dtype=np.float32)
    W = rng.standard_normal((DIM, DIM), dtype=np.float32) / math.sqrt(DIM)
    out = kernel(q, k, v, W)
    print(out.shape, out.dtype, np.abs(out).mean())


# revision 21
# speedup vs baseline: 15.3097x; 1.0208x over previous
"""Trainium2 Bass kernel: causal sliding-window attention + output projection.

Reference computation (B=1, H=16, T=2048, D=64, WINDOW=256, DIM=1024):
    att  = softmax(mask(q @ k^T / sqrt(D)))       per head, sliding causal window
    y    = att @ v                                 -> (B, H, T, D)
    out  = y.transpose -> (B, T, H*D) @ W_proj     -> (B, T, DIM)

Sharding over 8 NeuronCores: 2 head-groups (R) x 4 sequence-blocks (S).
Core c = (r, s): heads [8r, 8r+8), queries [512s, 512s+512), key window
[512s-256, 512s+512) (zero-padded below 0, NEG bias row kills padded keys).
W_proj row-sharded per head group; host sums the two partial projections per
sequence block (the "all-reduce after projection" done at gather time).

On-device layout (transposed so no on-chip transposes are needed):
  scores^T[k, q] = (kT_ext)^T @ qT_ext   65-row contraction: 64 dims + bias
                                         row giving -1e9 on padded keys.
  Each head is split into two half-stages (key tiles 0-2 / 3-5); each half is
  a [128, 768] f32 score tile living in a 2-bank PSUM slot so the attention
  PSUM footprint is sc(2x2 banks) + ot(2x1) = 6 banks, leaving 2 banks for
  the projection accumulators.  All tile pools persist across reps: rep N+1
  input DMAs and QK overlap rep N's projection, and there is no per-rep
  drain/barrier.
  P^T = exp(scores / 8)                  one ACT op per half [128, 768]
  3 chunked DVE multiplies per half apply the lo/hi triangle masks from a
  shared [128, 256] tri tile (strided access pattern pairs chunks).
  O = [ones | v]^T @ P^T                 ones|v read from one SBUF tile via a
                                         2-level AP: cols 0:64 are a memset
                                         ones block, v is contiguous after,
                                         so the v DMA lands with 6 KB
                                         contiguous elements per partition.
  rows 0:64 of O = softmax denominator, rows 64:128 = y^T.
  yT = O[64:128] / O[0:64]               one DVE tensor_tensor divide
  out[q, n] = sum_hp yT_hp^T @ W_hp      8 groups of 4 matmuls into 1-bank
                                         [128,512] PSUM tiles; copies to SBUF
                                         alternate ACT/DVE; DMA out per qt.
The PE stream runs one half-stage ahead of the AV consumers (QK of stage
s+1 is issued before AV of stage s) so the tensor engine stays busy and
ramps to its fast p-state.
"""

import math
import os
import sys

import numpy as np

for _p in ("/opt/trn_rl_repo",):
    if _p not in sys.path and os.path.isdir(_p):
        sys.path.insert(0, _p)

import ml_dtypes  # noqa: E402

BF16NP = ml_dtypes.bfloat16

B, H, T, D = 1, 16, 2048, 64
DIM = H * D
WINDOW = 256
R, S = 2, 4                 # head groups x sequence blocks
HL = H // R                 # 8 heads per core
QL = T // S                 # 512 queries per core
KW = QL + WINDOW            # 768-key window per core
NKT = KW // 128             # 6 key tiles
NQB = QL // 128             # 4 query blocks
NEG = -1.0e9                # additive bias for padded (out-of-range) keys
VWT_COLS = 8 * 6 * 128 + 4 * 1024 + 1536  # [ v | W | triA | triB ] combined

# Half-stage QK pieces: (kt, sc_col, q_col, width, start, stop).
# Score halves live in [128, 768] f32 inside a 2-bank (1024-col) PSUM slot;
# pieces never cross the 512-col bank line; start/stop mark first/last
# write per bank.
QK_A = [
    (0, 0, 0, 128, True, False),
    (1, 128, 0, 256, False, False),
    (2, 384, 0, 128, False, True),     # bank 0 done
    (2, 512, 128, 256, True, True),    # bank 1
]
QK_B = [
    (3, 0, 128, 256, True, False),
    (3, 256, 384, 128, False, False),
    (4, 384, 256, 128, False, True),   # bank 0 done
    (4, 512, 384, 128, True, False),   # bank 1
    (5, 640, 384, 128, False, True),
]

# AV pieces per half: (kt, pt_col, width, out_q_col).  Emission order within
# the whole head: A first (kt2 opens the ot accumulation group covering
# q0:384), B's kt5 is the only other fresh-write (q384:512); everything else
# accumulates, so the fresh-or-accumulate groups stay uniform per matmul.
AV_A = [
    (2, 384, 384, 0),       # start=True (opens group, q 0:384)
    (0, 0, 128, 0),
    (1, 128, 256, 0),
]
AV_B = [
    (5, 640, 128, 384),     # fresh write q384:512
    (3, 0, 384, 128),
    (4, 384, 256, 256),
]

# Mask ops per half: (pt_col, n_chunks, tri_col, engine).  Each op multiplies
# n_chunks 128-col chunks of pt (stride 128) by tri chunks starting at
# tri_col with stride 128.  tri = [lo | hi] (2 x 128 cols).  The 128-col ops
# go to the otherwise-idle GPSIMD; the 2-chunk op (which gates the
# group-opening AV matmul) stays on the faster DVE.
# half A chunks: [lo, one, lo, hi, one, lo] -> mask c0(lo), c2+c3(lo,hi), c5(lo)
# half B chunks: [hi, one, lo, hi, one, hi] -> mask c0(hi), c2+c3(lo,hi), c5(hi)
MASKS_A = [(256, 2, 0, "v"), (640, 1, 0, "g"), (0, 1, 0, "g")]
MASKS_B = [(256, 2, 0, "v"), (640, 1, 128, "g"), (0, 1, 128, "g")]

OPTS = {
    # one DVE divide would be ideal but the BIR verifier forbids two PSUM
    # inputs on TensorTensor, so default to recip+mul (one PSUM input each)
    "div_dve": False,
    "lookahead": 1,       # PE half-stages of QK lookahead before AV
    "pipeline_proj": True,  # interleave proj of rep N-1 into rep N attention
    "ncopies_act": 6,     # of the 8 proj copies, how many go to ACT
    "mask_full": True,    # one full-width mask op per half (else 3 chunked)
}


def _chunk_ap(AP, t, col, n):
    """[128, n, 128] AP over n 128-col chunks of t starting at col."""
    base = t[:, col:col + 128]
    return AP(base.tensor, base.offset,
              [list(base.ap[0]), [128, n], [1, 128]])


def _emit(tc, qkT_d, vwt_d, out_d, reps=1, opts=None):
    import concourse.mybir as mybir
    from concourse.bass import AP

    o = dict(OPTS)
    if opts:
        o.update(opts)
    nc = tc.nc
    BF16 = mybir.dt.bfloat16
    F32 = mybir.dt.float32
    Exp = mybir.ActivationFunctionType.Exp
    Copy = mybir.ActivationFunctionType.Copy
    Div = mybir.AluOpType.divide

    # ---- persistent pools (cross-rep pipelining, no per-rep barriers) ----
    from contextlib import ExitStack
    ctx = ExitStack()
    qk_pool = ctx.enter_context(tc.tile_pool(name="qk", bufs=2))
    v_pool = ctx.enter_context(tc.tile_pool(name="v", bufs=3))
    pt_pool = ctx.enter_context(tc.tile_pool(name="pt", bufs=4))
    yt_pool = ctx.enter_context(tc.tile_pool(name="yt", bufs=2))
    ost_pool = ctx.enter_context(tc.tile_pool(name="ost", bufs=2))
    sc_pool = ctx.enter_context(tc.tile_pool(name="sc", bufs=2, space="PSUM"))
    ot_pool = ctx.enter_context(tc.tile_pool(name="ot", bufs=2, space="PSUM"))
    op_pool = ctx.enter_context(tc.tile_pool(name="op", bufs=2, space="PSUM"))

    def emit_proj_group(g, yt_t, wt, ost_state):
        """Proj group g (qt = g//2, nh = g%2): 4 matmuls + copy; DMA per qt
        pair.  ost_state carries the current [128, 2048] staging tile."""
        qt, nh = g // 2, g % 2
        if g % 4 == 0:
            ost_state[0] = ost_pool.tile([128, 2048], BF16, tag="ost", name="ost")
        ost = ost_state[0]
        op_t = op_pool.tile([128, 512], F32, tag="op", name="op")
        for hp in range(4):
            nc.tensor.matmul(
                op_t[:],
                lhsT=yt_t[hp][:, qt * 128:(qt + 1) * 128],
                rhs=wt[:, hp * DIM + nh * 512:hp * DIM + nh * 512 + 512],
                start=(hp == 0), stop=(hp == 3),
            )
        dst = ost[:, (g % 4) * 512:(g % 4 + 1) * 512]
        if g % 8 < o["ncopies_act"]:
            nc.scalar.activation(dst, op_t[:], Copy)
        else:
            nc.vector.tensor_copy(dst, op_t[:])
        if g % 4 == 3:
            pair = g // 4
            src = AP(ost.tensor, ost.offset,
                     [list(ost.ap[0]), [1024, 2], [1, 1024]])
            dstd = AP(out_d.tensor, pair * 256 * DIM,
                      [[DIM, 128], [128 * DIM, 2], [1, DIM]])
            nc.sync.dma_start(dstd, src)

    def emit_inputs():
        """Allocate + DMA one rep's inputs.  Called one rep ahead so the
        transfers land before the consuming rep starts."""
        qk_t = qk_pool.tile([65, 4 * 2560], BF16, tag="qk", name="qk")
        nc.sync.dma_start(qk_t[:], qkT_d)
        # vwt holds [ v | W | tri ] in one DMA: v as [ones(64)|v(64)] per
        # (head, key-tile) interleaved in DRAM at pack time, so each AV lhsT
        # is a plain [128, 128] slice and the transfer is fully contiguous.
        vwt = v_pool.tile([128, VWT_COLS], BF16, tag="vwt", name="vwt")
        nc.sync.dma_start(vwt[:], vwt_d)
        return qk_t, vwt

    def one_rep(inputs, prev):
        qk_t, vwt = inputs
        vall = vwt[:, 0:8 * NKT * 128]
        wt = vwt[:, 8 * NKT * 128:8 * NKT * 128 + 4 * DIM]
        trib = 8 * NKT * 128 + 4 * DIM
        tri_half = [vwt[:, trib:trib + 768], vwt[:, trib + 768:trib + 1536]]

        def q_sl(h, a, b):
            base = (h // 2) * 2560 + (h % 2) * QL
            return qk_t[:, base + a:base + b]

        def k_sl(h, a, b):
            base = (h // 2) * 2560 + 2 * QL + (h % 2) * KW
            return qk_t[:, base + a:base + b]

        def v_lhsT(h, kt):
            b = (h * NKT + kt) * 128
            return vall[:, b:b + 128]

        yt_t = [
            yt_pool.tile([128, QL], BF16, tag=f"yt{hp}", name=f"yt{hp}")
            for hp in range(4)
        ]

        # ---- attention: 16 half-stages, PE runs `lookahead` ahead;
        # proj groups of the previous rep interleave every 2nd stage ----
        stages = [(h, half) for h in range(HL) for half in range(2)]
        pend = []          # (h, half, pt) awaiting AV
        ot_cur = [None]
        ost_state = [None]

        def emit_qk(h, half):
            sc = sc_pool.tile([128, 1024], F32, tag="sc", name="sc")
            table = QK_A if half == 0 else QK_B
            for (kt, so, qo, w, st, sp) in table:
                nc.tensor.matmul(
                    sc[:, so:so + w],
                    lhsT=k_sl(h, kt * 128, (kt + 1) * 128),
                    rhs=q_sl(h, qo, qo + w),
                    start=st, stop=sp,
                )
            pt = pt_pool.tile([128, 768], BF16, tag="pt", name="pt")
            nc.scalar.activation(pt[:], sc[:, 0:768], Exp, scale=1.0 / math.sqrt(D))
            if o["mask_full"]:
                # one full-width multiply per half; tri_half holds the
                # precomputed per-half [128, 768] lo/hi/one chunk pattern
                nc.vector.tensor_mul(pt[:], pt[:], tri_half[half])
            else:
                # chunked: skip the all-ones chunks (c1, c4); three DVE ops
                # of 128/256/128 cols against the packed lo|hi tri halves
                tri0 = 8 * NKT * 128 + 4 * DIM + half * 768
                for (pc, nck, tc_) in ((0, 1, 0), (256, 2, 256), (640, 1, 640)):
                    nc.vector.tensor_mul(
                        _chunk_ap(AP, pt, pc, nck),
                        _chunk_ap(AP, pt, pc, nck),
                        _chunk_ap(AP, vwt[:, tri0:tri0 + 768], tc_, nck),
                    )
            return pt

        def emit_av(h, half, pt):
            if half == 0:
                ot_cur[0] = ot_pool.tile([128, QL], F32, tag="ot", name="ot")
            ot = ot_cur[0]
            table = AV_A if half == 0 else AV_B
            for i, (kt, po, w, oq) in enumerate(table):
                nc.tensor.matmul(
                    ot[:, oq:oq + w],
                    lhsT=v_lhsT(h, kt),
                    rhs=pt[:, po:po + w],
                    start=(half == 0 and i == 0),
                    stop=(half == 1 and i == len(table) - 1),
                )
            if half == 1:
                hp, odd = h // 2, h % 2
                dst = yt_t[hp][odd * 64:(odd + 1) * 64, :]
                if o["div_dve"]:
                    nc.vector.tensor_tensor(
                        out=dst, in0=ot[64:128, :], in1=ot[0:64, :], op=Div)
                else:
                    rc = pt_pool.tile([64, QL], F32, tag="rc", name="rc")
                    nc.vector.reciprocal_approx_fast(out=rc[:], in_=ot[0:64, :])
                    nc.vector.tensor_mul(dst, ot[64:128, :], rc[:])

        la = o["lookahead"]
        for i, (h, half) in enumerate(stages):
            pend.append((h, half, emit_qk(h, half)))
            while len(pend) > la:
                ph, phalf, ppt = pend.pop(0)
                emit_av(ph, phalf, ppt)
            if prev is not None and i % 2 == 1:
                emit_proj_group(i // 2, prev[0], prev[1], ost_state)
        for (ph, phalf, ppt) in pend:
            emit_av(ph, phalf, ppt)

        return yt_t, wt

    prev = None
    pipeline = o["pipeline_proj"]
    cur_in = emit_inputs()
    for _rep in range(reps):
        nxt_in = emit_inputs() if _rep + 1 < reps else None
        cur = one_rep(cur_in, prev if pipeline else None)
        cur_in = nxt_in
        if pipeline:
            prev = cur
        else:
            ost_state = [None]
            for g in range(8):
                emit_proj_group(g, cur[0], cur[1], ost_state)
    if pipeline and prev is not None:
        ost_state = [None]
        for g in range(8):
            emit_proj_group(g, prev[0], prev[1], ost_state)
    ctx.close()


def build_program(reps=1, opts=None):
    """Build + compile the SPMD program once.  Returns the Bacc object."""
    from concourse import bacc, tile
    import concourse.mybir as mybir

    BF16 = mybir.dt.bfloat16

    nc = bacc.Bacc("TRN2", target_bir_lowering=False, debug=False, num_devices=8)
    qkT_d = nc.dram_tensor("qkT", [65, 4 * 2560], BF16, kind="ExternalInput").ap()
    vwt_d = nc.dram_tensor("vwt", [128, VWT_COLS], BF16, kind="ExternalInput").ap()
    out_d = nc.dram_tensor("out", [QL, DIM], BF16, kind="ExternalOutput").ap()

    with tile.TileContext(nc) as tc:
        _emit(tc, qkT_d, vwt_d, out_d, reps=reps, opts=opts)
    nc.compile()
    return nc


def pack_inputs(q, k, v, W_proj):
    """Shard + lay out the full inputs for the 8 cores.  Returns in_maps."""
    q = np.asarray(q, dtype=np.float32)
    k = np.asarray(k, dtype=np.float32)
    v = np.asarray(v, dtype=np.float32)
    W = np.asarray(W_proj, dtype=np.float32)

    p_idx = np.arange(128)[:, None]
    i_idx = np.arange(128)[None, :]
    lo = (p_idx > i_idx).astype(np.float32)
    hi = (p_idx <= i_idx).astype(np.float32)
    one = np.ones((128, 128), np.float32)
    triA = np.concatenate([lo, one, lo, hi, one, lo], axis=1)
    triB = np.concatenate([hi, one, lo, hi, one, hi], axis=1)
    tri_f32 = np.concatenate([triA, triB], axis=1)

    in_maps = []
    for c in range(8):
        r, s = c // S, c % S
        hs = slice(r * HL, (r + 1) * HL)
        qs = slice(s * QL, (s + 1) * QL)

        qh = q[0, hs, qs, :]                      # (HL, QL, D)
        qT = np.empty((HL, 65, QL), dtype=np.float32)
        qT[:, :64, :] = qh.transpose(0, 2, 1)
        qT[:, 64, :] = 1.0

        j0 = s * QL - WINDOW
        idx = j0 + np.arange(KW)
        valid = idx >= 0
        kh = np.zeros((HL, KW, D), dtype=np.float32)
        vh = np.zeros((HL, KW, D), dtype=np.float32)
        kh[:, valid] = k[0, hs][:, idx[valid], :]
        vh[:, valid] = v[0, hs][:, idx[valid], :]

        kT = np.empty((HL, 65, KW), dtype=np.float32)
        kT[:, :64, :] = kh.transpose(0, 2, 1)
        kT[:, 64, :] = np.where(valid, 0.0, NEG)[None, :]

        # qkT: per head-pair block [q h0 | q h1 | k h0 | k h1]
        qkT = np.empty((65, 4 * 2560), dtype=np.float32)
        for hp in range(4):
            b = hp * 2560
            qkT[:, b:b + QL] = qT[2 * hp]
            qkT[:, b + QL:b + 2 * QL] = qT[2 * hp + 1]
            qkT[:, b + 2 * QL:b + 2 * QL + KW] = kT[2 * hp]
            qkT[:, b + 2 * QL + KW:b + 2560] = kT[2 * hp + 1]

        # vE: per (head, key-tile) a [128, 128] block = [ones(64) | v(64)]
        vE = np.empty((128, 8 * NKT * 128), dtype=np.float32)
        for h in range(HL):
            for kt in range(NKT):
                b = (h * NKT + kt) * 128
                vE[:, b:b + 64] = 1.0
                vE[:, b + 64:b + 128] = vh[h, kt * 128:(kt + 1) * 128, :]

        Wc = np.ascontiguousarray(
            W[r * 512:(r + 1) * 512, :].reshape(4, 128, DIM)
        )
        Wc_b = np.ascontiguousarray(Wc.transpose(1, 0, 2).reshape(128, 4 * DIM))

        vwt = np.concatenate([vE, Wc_b, tri_f32], axis=1)

        in_maps.append({
            "qkT": qkT.astype(BF16NP),
            "vwt": vwt.astype(BF16NP),
        })
    return in_maps


def combine_outputs(results):
    """results[c]["out"] -> full (B, T, DIM) float32 output."""
    out = np.zeros((B, T, DIM), dtype=np.float32)
    for c in range(8):
        r, s = c // S, c % S
        out[0, s * QL:(s + 1) * QL, :] += np.asarray(
            results[c]["out"], dtype=np.float32)
    return out


_PROGRAM = None


def _get_program():
    global _PROGRAM
    if _PROGRAM is None:
        _PROGRAM = build_program()
    return _PROGRAM


def kernel(q, k, v, W_proj):
    from concourse.bass_utils import run_bass_kernel_spmd

    nc = _get_program()
    in_maps = pack_inputs(q, k, v, W_proj)
    res = run_bass_kernel_spmd(nc, in_maps, list(range(8)))
    return combine_outputs(res.results)


if __name__ == "__main__":
    # smoke test with random data
    rng = np.random.default_rng(0)
    q = rng.standard_normal((B, H, T, D), dtype=np.float32)
    k = rng.standard_normal((B, H, T, D), dtype=np.float32)
    v = rng.standard_normal((B, H, T, D), dtype=np.float32)
    W = rng.standard_normal((DIM, DIM), dtype=np.float32) / math.sqrt(DIM)
    out = kernel(q, k, v, W)
    print(out.shape, out.dtype, np.abs(out).mean())


# revision 22
# speedup vs baseline: 16.1070x; 1.0521x over previous
"""Trainium2 Bass kernel: causal sliding-window attention + output projection.

Reference computation (B=1, H=16, T=2048, D=64, WINDOW=256, DIM=1024):
    att  = softmax(mask(q @ k^T / sqrt(D)))       per head, sliding causal window
    y    = att @ v                                 -> (B, H, T, D)
    out  = y.transpose -> (B, T, H*D) @ W_proj     -> (B, T, DIM)

Sharding over 8 NeuronCores: 2 head-groups (R) x 4 sequence-blocks (S).
Core c = (r, s): heads [8r, 8r+8), queries [512s, 512s+512), key window
[512s-256, 512s+512) (zero-padded below 0, NEG bias row kills padded keys).
W_proj row-sharded per head group; host sums the two partial projections per
sequence block (the "all-reduce after projection" done at gather time).

On-device layout (transposed so no on-chip transposes are needed):
  scores^T[k, q] = (kT_ext)^T @ qT_ext   65-row contraction: 64 dims + bias
                                         row giving -1e9 on padded keys.
  Each head is split into two half-stages (key tiles 0-2 / 3-5); each half is
  a [128, 768] f32 score tile living in a 2-bank PSUM slot so the attention
  PSUM footprint is sc(2x2 banks) + ot(2x1) = 6 banks, leaving 2 banks for
  the projection accumulators.  All tile pools persist across reps: rep N+1
  input DMAs and QK overlap rep N's projection, and there is no per-rep
  drain/barrier.
  P^T = exp(scores / 8)                  one ACT op per half [128, 768]
  3 chunked DVE multiplies per half apply the lo/hi triangle masks from a
  shared [128, 256] tri tile (strided access pattern pairs chunks).
  O = [ones | v]^T @ P^T                 ones|v read from one SBUF tile via a
                                         2-level AP: cols 0:64 are a memset
                                         ones block, v is contiguous after,
                                         so the v DMA lands with 6 KB
                                         contiguous elements per partition.
  rows 0:64 of O = softmax denominator, rows 64:128 = y^T.
  yT = O[64:128] / O[0:64]               one DVE tensor_tensor divide
  out[q, n] = sum_hp yT_hp^T @ W_hp      8 groups of 4 matmuls into 1-bank
                                         [128,512] PSUM tiles; copies to SBUF
                                         alternate ACT/DVE; DMA out per qt.
The PE stream runs one half-stage ahead of the AV consumers (QK of stage
s+1 is issued before AV of stage s) so the tensor engine stays busy and
ramps to its fast p-state.
"""

import math
import os
import sys

import numpy as np

for _p in ("/opt/trn_rl_repo",):
    if _p not in sys.path and os.path.isdir(_p):
        sys.path.insert(0, _p)

import ml_dtypes  # noqa: E402

BF16NP = ml_dtypes.bfloat16

B, H, T, D = 1, 16, 2048, 64
DIM = H * D
WINDOW = 256
R, S = 2, 4                 # head groups x sequence blocks
HL = H // R                 # 8 heads per core
QL = T // S                 # 512 queries per core
KW = QL + WINDOW            # 768-key window per core
NKT = KW // 128             # 6 key tiles
NQB = QL // 128             # 4 query blocks
NEG = -1.0e9                # additive bias for padded (out-of-range) keys
VWT_COLS = 8 * 6 * 128 + 4 * 1024 + 1536  # [ v | W | triA | triB ] combined

# Half-stage QK pieces: (kt, sc_col, q_col, width, start, stop).
# Score halves live in [128, 768] f32 inside a 2-bank (1024-col) PSUM slot;
# pieces never cross the 512-col bank line; start/stop mark first/last
# write per bank.
QK_A = [
    (0, 0, 0, 128, True, False),
    (1, 128, 0, 256, False, False),
    (2, 384, 0, 128, False, True),     # bank 0 done
    (2, 512, 128, 256, True, True),    # bank 1
]
QK_B = [
    (3, 0, 128, 256, True, False),
    (3, 256, 384, 128, False, False),
    (4, 384, 256, 128, False, True),   # bank 0 done
    (4, 512, 384, 128, True, False),   # bank 1
    (5, 640, 384, 128, False, True),
]

# AV pieces per half: (kt, pt_col, width, out_q_col).  Emission order within
# the whole head: A first (kt2 opens the ot accumulation group covering
# q0:384), B's kt5 is the only other fresh-write (q384:512); everything else
# accumulates, so the fresh-or-accumulate groups stay uniform per matmul.
AV_A = [
    (2, 384, 384, 0),       # start=True (opens group, q 0:384)
    (0, 0, 128, 0),
    (1, 128, 256, 0),
]
AV_B = [
    (5, 640, 128, 384),     # fresh write q384:512
    (3, 0, 384, 128),
    (4, 384, 256, 256),
]

# Mask ops per half: (pt_col, n_chunks, tri_col, engine).  Each op multiplies
# n_chunks 128-col chunks of pt (stride 128) by tri chunks starting at
# tri_col with stride 128.  tri = [lo | hi] (2 x 128 cols).  The 128-col ops
# go to the otherwise-idle GPSIMD; the 2-chunk op (which gates the
# group-opening AV matmul) stays on the faster DVE.
# half A chunks: [lo, one, lo, hi, one, lo] -> mask c0(lo), c2+c3(lo,hi), c5(lo)
# half B chunks: [hi, one, lo, hi, one, hi] -> mask c0(hi), c2+c3(lo,hi), c5(hi)
MASKS_A = [(256, 2, 0, "v"), (640, 1, 0, "g"), (0, 1, 0, "g")]
MASKS_B = [(256, 2, 0, "v"), (640, 1, 128, "g"), (0, 1, 128, "g")]

OPTS = {
    # one DVE divide would be ideal but the BIR verifier forbids two PSUM
    # inputs on TensorTensor, so default to recip+mul (one PSUM input each)
    "div_dve": False,
    "lookahead": 1,       # PE half-stages of QK lookahead before AV
    "pipeline_proj": True,  # interleave proj of rep N-1 into rep N attention
    "ncopies_act": 6,     # of the 8 proj copies, how many go to ACT
    "mask_full": True,    # one full-width mask op per half (else 3 chunked)
}


def _chunk_ap(AP, t, col, n):
    """[128, n, 128] AP over n 128-col chunks of t starting at col."""
    base = t[:, col:col + 128]
    return AP(base.tensor, base.offset,
              [list(base.ap[0]), [128, n], [1, 128]])


def _emit(tc, qkT_d, vwt_d, out_d, reps=1, opts=None):
    import concourse.mybir as mybir
    from concourse.bass import AP

    o = dict(OPTS)
    if opts:
        o.update(opts)
    nc = tc.nc
    BF16 = mybir.dt.bfloat16
    F32 = mybir.dt.float32
    Exp = mybir.ActivationFunctionType.Exp
    Copy = mybir.ActivationFunctionType.Copy
    Div = mybir.AluOpType.divide

    # ---- persistent pools (cross-rep pipelining, no per-rep barriers) ----
    from contextlib import ExitStack
    ctx = ExitStack()
    qk_pool = ctx.enter_context(tc.tile_pool(name="qk", bufs=2))
    v_pool = ctx.enter_context(tc.tile_pool(name="v", bufs=3))
    pt_pool = ctx.enter_context(tc.tile_pool(name="pt", bufs=4))
    yt_pool = ctx.enter_context(tc.tile_pool(name="yt", bufs=2))
    ost_pool = ctx.enter_context(tc.tile_pool(name="ost", bufs=2))
    sc_pool = ctx.enter_context(tc.tile_pool(name="sc", bufs=2, space="PSUM"))
    ot_pool = ctx.enter_context(tc.tile_pool(name="ot", bufs=2, space="PSUM"))
    op_pool = ctx.enter_context(tc.tile_pool(name="op", bufs=2, space="PSUM"))

    def emit_proj_group(g, yt_t, wt, ost_state):
        """Proj group g (qt = g//2, nh = g%2): 4 matmuls + copy; DMA per qt
        pair.  ost_state carries the current [128, 2048] staging tile."""
        qt, nh = g // 2, g % 2
        if g % 4 == 0:
            ost_state[0] = ost_pool.tile([128, 2048], BF16, tag="ost", name="ost")
        ost = ost_state[0]
        op_t = op_pool.tile([128, 512], F32, tag="op", name="op")
        for hp in range(4):
            nc.tensor.matmul(
                op_t[:],
                lhsT=yt_t[hp][:, qt * 128:(qt + 1) * 128],
                rhs=wt[:, hp * DIM + nh * 512:hp * DIM + nh * 512 + 512],
                start=(hp == 0), stop=(hp == 3),
            )
        dst = ost[:, (g % 4) * 512:(g % 4 + 1) * 512]
        if g % 8 < o["ncopies_act"]:
            nc.scalar.activation(dst, op_t[:], Copy)
        else:
            nc.vector.tensor_copy(dst, op_t[:])
        if g % 4 == 3:
            pair = g // 4
            src = AP(ost.tensor, ost.offset,
                     [list(ost.ap[0]), [1024, 2], [1, 1024]])
            dstd = AP(out_d.tensor, pair * 256 * DIM,
                      [[DIM, 128], [128 * DIM, 2], [1, DIM]])
            nc.sync.dma_start(dstd, src)

    def emit_inputs():
        """Allocate + DMA one rep's inputs.  Called one rep ahead so the
        transfers land before the consuming rep starts."""
        qk_t = qk_pool.tile([65, 4 * 2560], BF16, tag="qk", name="qk")
        nc.sync.dma_start(qk_t[:], qkT_d)
        # vwt holds [ v | W | tri ] in one DMA: v as [ones(64)|v(64)] per
        # (head, key-tile) interleaved in DRAM at pack time, so each AV lhsT
        # is a plain [128, 128] slice and the transfer is fully contiguous.
        vwt = v_pool.tile([128, VWT_COLS], BF16, tag="vwt", name="vwt")
        nc.sync.dma_start(vwt[:], vwt_d)
        return qk_t, vwt

    def one_rep(inputs, prev):
        qk_t, vwt = inputs
        vall = vwt[:, 0:8 * NKT * 128]
        wt = vwt[:, 8 * NKT * 128:8 * NKT * 128 + 4 * DIM]
        trib = 8 * NKT * 128 + 4 * DIM
        tri_half = [vwt[:, trib:trib + 768], vwt[:, trib + 768:trib + 1536]]

        def q_sl(h, a, b):
            base = (h // 2) * 2560 + (h % 2) * QL
            return qk_t[:, base + a:base + b]

        def k_sl(h, a, b):
            base = (h // 2) * 2560 + 2 * QL + (h % 2) * KW
            return qk_t[:, base + a:base + b]

        def v_lhsT(h, kt):
            b = (h * NKT + kt) * 128
            return vall[:, b:b + 128]

        yt_t = [
            yt_pool.tile([128, QL], BF16, tag=f"yt{hp}", name=f"yt{hp}")
            for hp in range(4)
        ]

        # ---- attention: 16 half-stages, PE runs `lookahead` ahead;
        # proj groups of the previous rep interleave every 2nd stage ----
        stages = [(h, half) for h in range(HL) for half in range(2)]
        pend = []          # (h, half, pt) awaiting AV
        ot_cur = [None]
        ost_state = [None]

        def emit_qk(h, half):
            sc = sc_pool.tile([128, 1024], F32, tag="sc", name="sc")
            table = QK_A if half == 0 else QK_B
            for (kt, so, qo, w, st, sp) in table:
                nc.tensor.matmul(
                    sc[:, so:so + w],
                    lhsT=k_sl(h, kt * 128, (kt + 1) * 128),
                    rhs=q_sl(h, qo, qo + w),
                    start=st, stop=sp,
                )
            pt = pt_pool.tile([128, 768], BF16, tag="pt", name="pt")
            nc.scalar.activation(pt[:], sc[:, 0:768], Exp, scale=1.0 / math.sqrt(D))
            if o["mask_full"]:
                # two contiguous multiplies per half, upper part FIRST: the
                # ot-group-opening AV piece only needs pt[:, 384:768], so it
                # can start while the lower part is still being masked
                nc.vector.tensor_mul(pt[:, 384:768], pt[:, 384:768],
                                     tri_half[half][:, 384:768])
                nc.vector.tensor_mul(pt[:, 0:384], pt[:, 0:384],
                                     tri_half[half][:, 0:384])
            else:
                # chunked: skip the all-ones chunks (c1, c4); three DVE ops
                # of 128/256/128 cols against the packed lo|hi tri halves
                tri0 = 8 * NKT * 128 + 4 * DIM + half * 768
                for (pc, nck, tc_) in ((0, 1, 0), (256, 2, 256), (640, 1, 640)):
                    nc.vector.tensor_mul(
                        _chunk_ap(AP, pt, pc, nck),
                        _chunk_ap(AP, pt, pc, nck),
                        _chunk_ap(AP, vwt[:, tri0:tri0 + 768], tc_, nck),
                    )
            return pt

        def emit_av(h, half, pt):
            if half == 0:
                ot_cur[0] = ot_pool.tile([128, QL], F32, tag="ot", name="ot")
            ot = ot_cur[0]
            table = AV_A if half == 0 else AV_B
            for i, (kt, po, w, oq) in enumerate(table):
                nc.tensor.matmul(
                    ot[:, oq:oq + w],
                    lhsT=v_lhsT(h, kt),
                    rhs=pt[:, po:po + w],
                    start=(half == 0 and i == 0),
                    stop=(half == 1 and i == len(table) - 1),
                )
            if half == 1:
                hp, odd = h // 2, h % 2
                dst = yt_t[hp][odd * 64:(odd + 1) * 64, :]
                if o["div_dve"]:
                    nc.vector.tensor_tensor(
                        out=dst, in0=ot[64:128, :], in1=ot[0:64, :], op=Div)
                else:
                    rc = pt_pool.tile([64, QL], F32, tag="rc", name="rc")
                    nc.vector.reciprocal_approx_fast(out=rc[:], in_=ot[0:64, :])
                    nc.vector.tensor_mul(dst, ot[64:128, :], rc[:])

        la = o["lookahead"]
        for i, (h, half) in enumerate(stages):
            pend.append((h, half, emit_qk(h, half)))
            while len(pend) > la:
                ph, phalf, ppt = pend.pop(0)
                emit_av(ph, phalf, ppt)
            if prev is not None and i % 2 == 1:
                emit_proj_group(i // 2, prev[0], prev[1], ost_state)
        for (ph, phalf, ppt) in pend:
            emit_av(ph, phalf, ppt)

        return yt_t, wt

    prev = None
    pipeline = o["pipeline_proj"]
    cur_in = emit_inputs()
    for _rep in range(reps):
        nxt_in = emit_inputs() if _rep + 1 < reps else None
        cur = one_rep(cur_in, prev if pipeline else None)
        cur_in = nxt_in
        if pipeline:
            prev = cur
        else:
            ost_state = [None]
            for g in range(8):
                emit_proj_group(g, cur[0], cur[1], ost_state)
    if pipeline and prev is not None:
        ost_state = [None]
        for g in range(8):
            emit_proj_group(g, prev[0], prev[1], ost_state)
    ctx.close()


def build_program(reps=1, opts=None):
    """Build + compile the SPMD program once.  Returns the Bacc object."""
    from concourse import bacc, tile
    import concourse.mybir as mybir

    BF16 = mybir.dt.bfloat16

    nc = bacc.Bacc("TRN2", target_bir_lowering=False, debug=False, num_devices=8)
    qkT_d = nc.dram_tensor("qkT", [65, 4 * 2560], BF16, kind="ExternalInput").ap()
    vwt_d = nc.dram_tensor("vwt", [128, VWT_COLS], BF16, kind="ExternalInput").ap()
    out_d = nc.dram_tensor("out", [QL, DIM], BF16, kind="ExternalOutput").ap()

    with tile.TileContext(nc) as tc:
        _emit(tc, qkT_d, vwt_d, out_d, reps=reps, opts=opts)
    nc.compile()
    return nc


def pack_inputs(q, k, v, W_proj):
    """Shard + lay out the full inputs for the 8 cores.  Returns in_maps."""
    q = np.asarray(q, dtype=np.float32)
    k = np.asarray(k, dtype=np.float32)
    v = np.asarray(v, dtype=np.float32)
    W = np.asarray(W_proj, dtype=np.float32)

    p_idx = np.arange(128)[:, None]
    i_idx = np.arange(128)[None, :]
    lo = (p_idx > i_idx).astype(np.float32)
    hi = (p_idx <= i_idx).astype(np.float32)
    one = np.ones((128, 128), np.float32)
    triA = np.concatenate([lo, one, lo, hi, one, lo], axis=1)
    triB = np.concatenate([hi, one, lo, hi, one, hi], axis=1)
    tri_f32 = np.concatenate([triA, triB], axis=1)

    in_maps = []
    for c in range(8):
        r, s = c // S, c % S
        hs = slice(r * HL, (r + 1) * HL)
        qs = slice(s * QL, (s + 1) * QL)

        qh = q[0, hs, qs, :]                      # (HL, QL, D)
        qT = np.empty((HL, 65, QL), dtype=np.float32)
        qT[:, :64, :] = qh.transpose(0, 2, 1)
        qT[:, 64, :] = 1.0

        j0 = s * QL - WINDOW
        idx = j0 + np.arange(KW)
        valid = idx >= 0
        kh = np.zeros((HL, KW, D), dtype=np.float32)
        vh = np.zeros((HL, KW, D), dtype=np.float32)
        kh[:, valid] = k[0, hs][:, idx[valid], :]
        vh[:, valid] = v[0, hs][:, idx[valid], :]

        kT = np.empty((HL, 65, KW), dtype=np.float32)
        kT[:, :64, :] = kh.transpose(0, 2, 1)
        kT[:, 64, :] = np.where(valid, 0.0, NEG)[None, :]

        # qkT: per head-pair block [q h0 | q h1 | k h0 | k h1]
        qkT = np.empty((65, 4 * 2560), dtype=np.float32)
        for hp in range(4):
            b = hp * 2560
            qkT[:, b:b + QL] = qT[2 * hp]
            qkT[:, b + QL:b + 2 * QL] = qT[2 * hp + 1]
            qkT[:, b + 2 * QL:b + 2 * QL + KW] = kT[2 * hp]
            qkT[:, b + 2 * QL + KW:b + 2560] = kT[2 * hp + 1]

        # vE: per (head, key-tile) a [128, 128] block = [ones(64) | v(64)]
        vE = np.empty((128, 8 * NKT * 128), dtype=np.float32)
        for h in range(HL):
            for kt in range(NKT):
                b = (h * NKT + kt) * 128
                vE[:, b:b + 64] = 1.0
                vE[:, b + 64:b + 128] = vh[h, kt * 128:(kt + 1) * 128, :]

        Wc = np.ascontiguousarray(
            W[r * 512:(r + 1) * 512, :].reshape(4, 128, DIM)
        )
        Wc_b = np.ascontiguousarray(Wc.transpose(1, 0, 2).reshape(128, 4 * DIM))

        vwt = np.concatenate([vE, Wc_b, tri_f32], axis=1)

        in_maps.append({
            "qkT": qkT.astype(BF16NP),
            "vwt": vwt.astype(BF16NP),
        })
    return in_maps


def combine_outputs(results):
    """results[c]["out"] -> full (B, T, DIM) float32 output."""
    out = np.zeros((B, T, DIM), dtype=np.float32)
    for c in range(8):
        r, s = c // S, c % S
        out[0, s * QL:(s + 1) * QL, :] += np.asarray(
            results[c]["out"], dtype=np.float32)
    return out


_PROGRAM = None


def _get_program():
    global _PROGRAM
    if _PROGRAM is None:
        _PROGRAM = build_program()
    return _PROGRAM


def kernel(q, k, v, W_proj):
    from concourse.bass_utils import run_bass_kernel_spmd

    nc = _get_program()
    in_maps = pack_inputs(q, k, v, W_proj)
    res = run_bass_kernel_spmd(nc, in_maps, list(range(8)))
    return combine_outputs(res.results)


if __name__ == "__main__":
    # smoke test with random data
    rng = np.random.default_rng(0)
    q = rng.standard_normal((B, H, T, D), dtype=np.float32)
    k = rng.standard_normal((B, H, T, D), dtype=np.float32)
    v = rng.standard_normal((B, H, T, D), dtype=np.float32)
    W = rng.standard_normal((DIM, DIM), dtype=np.float32) / math.sqrt(DIM)
    out = kernel(q, k, v, W)
    print(out.shape, out.dtype, np.abs(out).mean())
